# revision 8
# baseline (speedup 1.0000x reference)
"""Trainium2 Bass kernel for MemoryEfficientCrossAttention.

Reference computation (per batch b):
    q = x @ Wq, k = x @ Wk, v = x @ Wv          (heads split from inner dim)
    scores = (q_h @ k_h^T) / sqrt(D)
    out_h  = softmax(scores) @ v_h
    y      = concat_h(out_h) @ Wo + bo

Shapes: x [2, 4096, 320], W* [320, 512], Wo [512, 320], H=8 heads, D=64.

Sharding: 16 (batch, head) problems over 8 cores -> each core gets one
batch and two adjacent heads. Host pre-transposes x, slices the weight
columns for the core's two heads (folding the 1/sqrt(D) scale into Wq),
and sums the per-core partial outputs (each core contributes its two
heads' slice of the final projection).

Per-core layout (A = first head, B = second head):
  qT/kT in SBUF as [128, S]: partitions 0:64 hold head A's [d, s],
  64:128 head B's -> the two heads' score matmuls run as concurrent
  row-tiled PE pairs (contract dim is D=64). Scores land transposed
  [k_tile=128, q] in PSUM, exp runs on ScalarE over a [128, 2048] PSUM
  window (both heads), and AV uses stationary [V | ones] so the softmax
  denominator accumulates as psum row 64 of the [65, q] output. DVE
  stream-shuffle broadcasts the reciprocal row for normalization.
"""

import sys

sys.path.insert(0, "/opt/trn_rl_repo")

import numpy as np

import concourse.bass as bass
import concourse.mybir as mybir
import concourse.tile as tile
from concourse import bacc
from concourse.bass_utils import run_bass_kernel_spmd

B, S, QD = 2, 4096, 320
H, D = 8, 64
QDP = 384  # QD padded to 3 chunks of 128
F32 = mybir.dt.float32
F32R = mybir.dt.float32r


def build_nc(s=S, qb=1024, trace_sim=False):
    """Build the per-core Bass program. s = sequence length (small for sim
    tests), qb = query block (columns of scores produced per PE/ACT round).
    """
    n_kt = s // 128  # key tiles
    n_qb = s // qb  # query blocks
    n_sb = max(s // 512, 1)  # 512-wide column blocks for the projections
    sbw = min(s, 512)
    n_qt = s // 128  # query tiles (output projection)
    assert qb % 512 == 0 or qb == s

    nc = bacc.Bacc("TRN2", target_bir_lowering=False, debug=False, num_devices=8)
    xT = nc.dram_tensor("xT", [QDP, s], F32R, kind="ExternalInput").ap()
    wq = nc.dram_tensor("wq", [QDP, 128], F32R, kind="ExternalInput").ap()
    wk = nc.dram_tensor("wk", [QDP, 128], F32R, kind="ExternalInput").ap()
    wv = nc.dram_tensor("wv", [QDP, 128], F32R, kind="ExternalInput").ap()
    wo = nc.dram_tensor("wo", [128, QD], F32R, kind="ExternalInput").ap()
    y = nc.dram_tensor("y", [s, QD], F32, kind="ExternalOutput").ap()

    with tile.TileContext(nc, trace_sim=trace_sim) as tc:
        with (
            tc.tile_pool(name="persist", bufs=1) as persist,
            tc.tile_pool(name="wpool", bufs=1) as wpool,
        ):
            # ---- load inputs -------------------------------------------------
            xT_sb = persist.tile([128, 3, s], F32R)
            xT_r = xT.rearrange("(c p) s -> c p s", p=128)
            for c in range(3):
                nc.sync.dma_start(out=xT_sb[:, c, :], in_=xT_r[c])
            w_sb = {}
            for name, w in (("wq", wq), ("wk", wk), ("wv", wv)):
                w_sb[name] = wpool.tile([128, 3, 128], F32R, name=f"{name}_sb")
                w_r = w.rearrange("(c p) m -> c p m", p=128)
                for c in range(3):
                    nc.sync.dma_start(out=w_sb[name][:, c, :], in_=w_r[c])
            wo_sb = wpool.tile([128, QD], F32R)
            nc.sync.dma_start(out=wo_sb, in_=wo)

            qT_sb = persist.tile([128, s], F32R)  # rows 0:64 head A, 64:128 head B
            kT_sb = persist.tile([128, s], F32R)
            v_sb = persist.tile([128, 2, n_kt, 65], F32R)  # [k, head, ktile, d|1]
            outT_sb = persist.tile([128, s], F32R)  # normalized attn out, transposed

            for h in range(2):
                nc.vector.memset(v_sb[:, h, :, 64].bitcast(F32), 1.0)

            # ---- projections -------------------------------------------------
            with tc.tile_pool(name="proj_ps", bufs=3, space="PSUM") as proj_ps:
                for name, dst in (("wq", qT_sb), ("wk", kT_sb)):
                    for sb in range(n_sb):
                        ps = proj_ps.tile([128, sbw], F32, tag="qk")
                        for c in range(3):
                            nc.tensor.matmul(
                                ps,
                                w_sb[name][:, c, :],
                                xT_sb[:, c, sb * sbw : (sb + 1) * sbw],
                                start=(c == 0),
                                stop=(c == 2),
                            )
                        nc.vector.tensor_copy(dst[:, sb * sbw : (sb + 1) * sbw], ps)
                for st in range(n_kt):
                    vp = proj_ps.tile([128, 128], F32, tag="v")
                    for c in range(3):
                        nc.tensor.matmul(
                            vp,
                            xT_sb[:, c, st * 128 : (st + 1) * 128],
                            w_sb["wv"][:, c, :],
                            start=(c == 0),
                            stop=(c == 2),
                        )
                    for h in range(2):
                        nc.vector.tensor_copy(
                            v_sb[:, h, st, 0:64], vp[:, h * 64 : (h + 1) * 64]
                        )

            # ---- attention ---------------------------------------------------
            with (
                tc.tile_pool(name="sc_ps", bufs=1, space="PSUM") as sc_ps,
                tc.tile_pool(name="av_ps", bufs=1, space="PSUM") as av_ps,
                tc.tile_pool(name="exp_sb", bufs=3) as exp_pool,
                tc.tile_pool(name="norm_sb", bufs=2) as norm_pool,
            ):
                n_j = qb // 512 if qb >= 512 else 1
                jw = min(qb, 512)
                for q0 in range(n_qb):
                    qsl = slice(q0 * qb, (q0 + 1) * qb)
                    av = [
                        av_ps.tile([128, qb], F32, name=f"av{h}", tag=f"av{h}")
                        for h in range(2)
                    ]
                    for kt in range(n_kt):
                        sc = sc_ps.tile([128, 2 * qb], F32, tag="sc")
                        for h in range(2):
                            hp = slice(h * 64, (h + 1) * 64)
                            for j in range(n_j):
                                nc.tensor.matmul(
                                    sc[:, h * qb + j * jw : h * qb + (j + 1) * jw],
                                    kT_sb[hp, kt * 128 : (kt + 1) * 128],
                                    qT_sb[hp, q0 * qb + j * jw : q0 * qb + (j + 1) * jw],
                                    start=True,
                                    stop=True,
                                )
                        ex = exp_pool.tile([128, 2 * qb], F32R, tag="ex")
                        nc.scalar.activation(ex, sc, mybir.ActivationFunctionType.Exp)
                        for h in range(2):
                            for j in range(n_j):
                                nc.tensor.matmul(
                                    av[h][0:65, j * jw : (j + 1) * jw],
                                    v_sb[:, h, kt, :],
                                    ex[:, h * qb + j * jw : h * qb + (j + 1) * jw],
                                    start=(kt == 0),
                                    stop=(kt == n_kt - 1),
                                )
                    # normalize: out[d, q] = av[d, q] / av[64, q]
                    for h in range(2):
                        rrow = norm_pool.tile([1, qb], F32, tag="rrow")
                        nc.vector.reciprocal(rrow, av[h][64:65, :])
                        rc = norm_pool.tile([64, qb], F32, tag="rc")
                        nc.gpsimd.partition_broadcast(rc, rrow)
                        nc.vector.tensor_mul(
                            outT_sb[h * 64 : (h + 1) * 64, qsl], av[h][0:64, :], rc
                        )

            # ---- output projection ------------------------------------------
            with (
                tc.tile_pool(name="fin_ps", bufs=3, space="PSUM") as fin_ps,
                tc.tile_pool(name="y_sb", bufs=3) as y_pool,
            ):
                for qt in range(n_qt):
                    fp = fin_ps.tile([128, QD], F32, tag="fin")
                    qtsl = slice(qt * 128, (qt + 1) * 128)
                    # contract over all 128 partitions: rows 0:64 are head A's
                    # dims against Wo_A, 64:128 head B's against Wo_B -> the
                    # two heads' contributions sum in one matmul.
                    nc.tensor.matmul(
                        fp, outT_sb[:, qtsl], wo_sb, start=True, stop=True
                    )
                    yt = y_pool.tile([128, QD], F32, tag="y")
                    nc.vector.tensor_copy(yt, fp)
                    nc.sync.dma_start(out=y[qtsl, :], in_=yt)

    nc.compile()
    return nc


def host_inputs(x, Wq, Wk, Wv, Wo):
    """Build the 8 per-core input dicts from the full problem inputs."""
    scale = 1.0 / np.sqrt(np.float32(D))
    in_maps = []
    for core in range(8):
        b = core // 4
        h0 = 2 * (core % 4)
        cols = slice(h0 * D, (h0 + 2) * D)
        xT = np.zeros((QDP, S), np.float32)
        xT[:QD] = np.ascontiguousarray(x[b].T)
        wq_c = np.zeros((QDP, 128), np.float32)
        wq_c[:QD] = Wq[:, cols] * scale
        wk_c = np.zeros((QDP, 128), np.float32)
        wk_c[:QD] = Wk[:, cols]
        wv_c = np.zeros((QDP, 128), np.float32)
        wv_c[:QD] = Wv[:, cols]
        wo_c = np.ascontiguousarray(Wo[cols, :], dtype=np.float32)
        in_maps.append({"xT": xT, "wq": wq_c, "wk": wk_c, "wv": wv_c, "wo": wo_c})
    return in_maps


_NC_CACHE = {}


def kernel(x, Wq, Wk, Wv, Wo, bo, _trace=False, _trace_kwargs=None):
    x = np.asarray(x, np.float32)
    Wq = np.asarray(Wq, np.float32)
    Wk = np.asarray(Wk, np.float32)
    Wv = np.asarray(Wv, np.float32)
    Wo = np.asarray(Wo, np.float32)
    bo = np.asarray(bo, np.float32)

    if "nc" not in _NC_CACHE:
        _NC_CACHE["nc"] = build_nc()
    nc = _NC_CACHE["nc"]

    in_maps = host_inputs(x, Wq, Wk, Wv, Wo)
    kwargs = {}
    if _trace:
        kwargs = dict(trace=True, **(_trace_kwargs or {}))
    res = run_bass_kernel_spmd(nc, in_maps, core_ids=list(range(8)), **kwargs)

    y = np.zeros((B, S, QD), np.float32)
    for core in range(8):
        y[core // 4] += res.results[core]["y"]
    y += bo
    if _trace:
        return y, res
    return y


# revision 9
# speedup vs baseline: 1.9094x; 1.9094x over previous
"""Trainium2 Bass kernel for MemoryEfficientCrossAttention.

Reference computation (per batch b):
    q = x @ Wq, k = x @ Wk, v = x @ Wv          (heads split from inner dim)
    scores = (q_h @ k_h^T) / sqrt(D)
    out_h  = softmax(scores) @ v_h
    y      = concat_h(out_h) @ Wo + bo

Shapes: x [2, 4096, 320], W* [320, 512], Wo [512, 320], H=8 heads, D=64.

Sharding: 16 (batch, head) problems over 8 cores -> each core gets one
batch and two adjacent heads. Host pre-transposes x, slices the weight
columns for the core's two heads (folding the 1/sqrt(D) scale into Wq),
and sums the per-core partial outputs (each core contributes its two
heads' slice of the final projection).

Per-core layout (A = first head, B = second head):
  qT/kT in SBUF as [128, S]: partitions 0:64 hold head A's [d, s],
  64:128 head B's -> the two heads' score matmuls run as concurrent
  row-tiled PE pairs (contract dim is D=64). Scores land transposed
  [k_tile=128, q] in PSUM, exp runs on ScalarE over a [128, 2048] PSUM
  window (both heads), and AV uses stationary [V | ones] so the softmax
  denominator accumulates as psum row 64 of the [65, q] output. DVE
  stream-shuffle broadcasts the reciprocal row for normalization.
"""

import sys

sys.path.insert(0, "/opt/trn_rl_repo")

import numpy as np

import concourse.bass as bass
import concourse.mybir as mybir
import concourse.tile as tile
from concourse import bacc
from concourse.bass_utils import run_bass_kernel_spmd

B, S, QD = 2, 4096, 320
H, D = 8, 64
QDP = 384  # QD padded to 3 chunks of 128
F32 = mybir.dt.float32
F32R = mybir.dt.float32r


def build_nc(s=S, qb=512, trace_sim=False):
    """Build the per-core Bass program. s = sequence length (small for sim
    tests), qb = query block (columns of scores produced per PE/ACT round).
    """
    n_kt = s // 128  # key tiles
    n_qb = s // qb  # query blocks
    n_sb = max(s // 512, 1)  # 512-wide column blocks for the projections
    sbw = min(s, 512)
    n_qt = s // 128  # query tiles (output projection)
    assert qb % 512 == 0 or qb == s

    nc = bacc.Bacc("TRN2", target_bir_lowering=False, debug=False, num_devices=8)
    xT = nc.dram_tensor("xT", [QDP, s], F32R, kind="ExternalInput").ap()
    wq = nc.dram_tensor("wq", [QDP, 128], F32R, kind="ExternalInput").ap()
    wk = nc.dram_tensor("wk", [QDP, 128], F32R, kind="ExternalInput").ap()
    wv = nc.dram_tensor("wv", [QDP, 128], F32R, kind="ExternalInput").ap()
    wo = nc.dram_tensor("wo", [128, QD], F32R, kind="ExternalInput").ap()
    y = nc.dram_tensor("y", [s, QD], F32, kind="ExternalOutput").ap()

    with tile.TileContext(nc, trace_sim=trace_sim) as tc:
        with (
            tc.tile_pool(name="persist", bufs=1) as persist,
            tc.tile_pool(name="wpool", bufs=1) as wpool,
        ):
            # ---- load inputs -------------------------------------------------
            xT_sb = persist.tile([128, 3, s], F32R)
            xT_r = xT.rearrange("(c p) s -> c p s", p=128)
            for c in range(3):
                nc.sync.dma_start(out=xT_sb[:, c, :], in_=xT_r[c])
            w_sb = {}
            for name, w in (("wq", wq), ("wk", wk), ("wv", wv)):
                w_sb[name] = wpool.tile([128, 3, 128], F32R, name=f"{name}_sb")
                w_r = w.rearrange("(c p) m -> c p m", p=128)
                for c in range(3):
                    nc.sync.dma_start(out=w_sb[name][:, c, :], in_=w_r[c])
            wo_sb = wpool.tile([128, QD], F32R)
            nc.sync.dma_start(out=wo_sb, in_=wo)

            qT_sb = persist.tile([128, s], F32R)  # rows 0:64 head A, 64:128 head B
            kT_sb = persist.tile([128, s], F32R)
            v_sb = persist.tile([128, 2, n_kt, 65], F32R)  # [k, head, ktile, d|1]
            outT_sb = persist.tile([128, s], F32R)  # normalized attn out, transposed

            for h in range(2):
                nc.vector.memset(v_sb[:, h, :, 64].bitcast(F32), 1.0)

            # bf16 warmup burst: ~20 matmuls of dense PE work during the
            # input DMAs so the HAM clock-gate reaches K=8/8 (2.4 GHz)
            # before the projections start. fp32r keeps warmth but the
            # cold-start is cheaper in bf16.
            with tc.tile_pool(name="warm_ps", bufs=1, space="PSUM") as warm_ps:
                wt = persist.tile([128, 512], mybir.dt.bfloat16, name="warm_t")
                nc.vector.memset(wt, 0.25)
                wp = warm_ps.tile([128, 512], F32)
                for i in range(20):
                    nc.tensor.matmul(
                        wp, wt[:, 0:128], wt, start=True, stop=True
                    )

            # ---- projections -------------------------------------------------
            with tc.tile_pool(name="proj_ps", bufs=3, space="PSUM") as proj_ps:
                for name, dst in (("wq", qT_sb), ("wk", kT_sb)):
                    for sb in range(n_sb):
                        ps = proj_ps.tile([128, sbw], F32, tag="qk")
                        for c in range(3):
                            nc.tensor.matmul(
                                ps,
                                w_sb[name][:, c, :],
                                xT_sb[:, c, sb * sbw : (sb + 1) * sbw],
                                start=(c == 0),
                                stop=(c == 2),
                            )
                        nc.vector.tensor_copy(dst[:, sb * sbw : (sb + 1) * sbw], ps)
                for st in range(n_kt):
                    vp = proj_ps.tile([128, 128], F32, tag="v")
                    for c in range(3):
                        nc.tensor.matmul(
                            vp,
                            xT_sb[:, c, st * 128 : (st + 1) * 128],
                            w_sb["wv"][:, c, :],
                            start=(c == 0),
                            stop=(c == 2),
                        )
                    for h in range(2):
                        nc.vector.tensor_copy(
                            v_sb[:, h, st, 0:64], vp[:, h * 64 : (h + 1) * 64]
                        )

            # ---- attention ---------------------------------------------------
            with (
                tc.tile_pool(name="sc_ps", bufs=2, space="PSUM") as sc_ps,
                tc.tile_pool(name="av_ps", bufs=2, space="PSUM") as av_ps,
                tc.tile_pool(name="exp_sb", bufs=3) as exp_pool,
                tc.tile_pool(name="norm_sb", bufs=2) as norm_pool,
            ):
                n_j = qb // 512 if qb >= 512 else 1
                jw = min(qb, 512)
                for q0 in range(n_qb):
                    qsl = slice(q0 * qb, (q0 + 1) * qb)
                    av = [
                        av_ps.tile([128, qb], F32, name=f"av{h}", tag=f"av{h}")
                        for h in range(2)
                    ]
                    for kt in range(n_kt):
                        sc = sc_ps.tile([128, 2 * qb], F32, tag="sc")
                        for h in range(2):
                            hp = slice(h * 64, (h + 1) * 64)
                            for j in range(n_j):
                                nc.tensor.matmul(
                                    sc[:, h * qb + j * jw : h * qb + (j + 1) * jw],
                                    kT_sb[hp, kt * 128 : (kt + 1) * 128],
                                    qT_sb[hp, q0 * qb + j * jw : q0 * qb + (j + 1) * jw],
                                    start=True,
                                    stop=True,
                                )
                        ex = exp_pool.tile([128, 2 * qb], F32R, tag="ex")
                        nc.scalar.activation(ex, sc, mybir.ActivationFunctionType.Exp)
                        for h in range(2):
                            for j in range(n_j):
                                nc.tensor.matmul(
                                    av[h][0:65, j * jw : (j + 1) * jw],
                                    v_sb[:, h, kt, :],
                                    ex[:, h * qb + j * jw : h * qb + (j + 1) * jw],
                                    start=(kt == 0),
                                    stop=(kt == n_kt - 1),
                                )
                    # normalize: out[d, q] = av[d, q] / av[64, q]
                    for h in range(2):
                        rrow = norm_pool.tile([1, qb], F32, tag="rrow")
                        nc.vector.reciprocal(rrow, av[h][64:65, :])
                        rc = norm_pool.tile([64, qb], F32, tag="rc")
                        nc.gpsimd.partition_broadcast(rc, rrow)
                        nc.vector.tensor_mul(
                            outT_sb[h * 64 : (h + 1) * 64, qsl], av[h][0:64, :], rc
                        )

            # ---- output projection ------------------------------------------
            with (
                tc.tile_pool(name="fin_ps", bufs=3, space="PSUM") as fin_ps,
                tc.tile_pool(name="y_sb", bufs=3) as y_pool,
            ):
                for qt in range(n_qt):
                    fp = fin_ps.tile([128, QD], F32, tag="fin")
                    qtsl = slice(qt * 128, (qt + 1) * 128)
                    # contract over all 128 partitions: rows 0:64 are head A's
                    # dims against Wo_A, 64:128 head B's against Wo_B -> the
                    # two heads' contributions sum in one matmul.
                    nc.tensor.matmul(
                        fp, outT_sb[:, qtsl], wo_sb, start=True, stop=True
                    )
                    yt = y_pool.tile([128, QD], F32, tag="y")
                    nc.vector.tensor_copy(yt, fp)
                    nc.sync.dma_start(out=y[qtsl, :], in_=yt)

    nc.compile()
    return nc


def host_inputs(x, Wq, Wk, Wv, Wo):
    """Build the 8 per-core input dicts from the full problem inputs."""
    scale = 1.0 / np.sqrt(np.float32(D))
    in_maps = []
    for core in range(8):
        b = core // 4
        h0 = 2 * (core % 4)
        cols = slice(h0 * D, (h0 + 2) * D)
        xT = np.zeros((QDP, S), np.float32)
        xT[:QD] = np.ascontiguousarray(x[b].T)
        wq_c = np.zeros((QDP, 128), np.float32)
        wq_c[:QD] = Wq[:, cols] * scale
        wk_c = np.zeros((QDP, 128), np.float32)
        wk_c[:QD] = Wk[:, cols]
        wv_c = np.zeros((QDP, 128), np.float32)
        wv_c[:QD] = Wv[:, cols]
        wo_c = np.ascontiguousarray(Wo[cols, :], dtype=np.float32)
        in_maps.append({"xT": xT, "wq": wq_c, "wk": wk_c, "wv": wv_c, "wo": wo_c})
    return in_maps


_NC_CACHE = {}


def kernel(x, Wq, Wk, Wv, Wo, bo, _trace=False, _trace_kwargs=None):
    x = np.asarray(x, np.float32)
    Wq = np.asarray(Wq, np.float32)
    Wk = np.asarray(Wk, np.float32)
    Wv = np.asarray(Wv, np.float32)
    Wo = np.asarray(Wo, np.float32)
    bo = np.asarray(bo, np.float32)

    if "nc" not in _NC_CACHE:
        _NC_CACHE["nc"] = build_nc()
    nc = _NC_CACHE["nc"]

    in_maps = host_inputs(x, Wq, Wk, Wv, Wo)
    kwargs = {}
    if _trace:
        kwargs = dict(trace=True, **(_trace_kwargs or {}))
    res = run_bass_kernel_spmd(nc, in_maps, core_ids=list(range(8)), **kwargs)

    y = np.zeros((B, S, QD), np.float32)
    for core in range(8):
        y[core // 4] += res.results[core]["y"]
    y += bo
    if _trace:
        return y, res
    return y


# revision 10
# speedup vs baseline: 1.9255x; 1.0084x over previous
"""Trainium2 Bass kernel for MemoryEfficientCrossAttention.

Reference computation (per batch b):
    q = x @ Wq, k = x @ Wk, v = x @ Wv          (heads split from inner dim)
    scores = (q_h @ k_h^T) / sqrt(D)
    out_h  = softmax(scores) @ v_h
    y      = concat_h(out_h) @ Wo + bo

Shapes: x [2, 4096, 320], W* [320, 512], Wo [512, 320], H=8 heads, D=64.

Sharding: 16 (batch, head) problems over 8 cores -> each core gets one
batch and two adjacent heads. Host pre-transposes x, slices the weight
columns for the core's two heads (folding the 1/sqrt(D) scale into Wq),
and sums the per-core partial outputs (each core contributes its two
heads' slice of the final projection).

Per-core layout (A = first head, B = second head):
  qT/kT in SBUF as [128, S]: partitions 0:64 hold head A's [d, s],
  64:128 head B's -> the two heads' score matmuls run as concurrent
  row-tiled PE pairs (contract dim is D=64). Scores land transposed
  [k_tile=128, q] in PSUM, exp runs on ScalarE over a [128, 2048] PSUM
  window (both heads), and AV uses stationary [V | ones] so the softmax
  denominator accumulates as psum row 64 of the [65, q] output. DVE
  stream-shuffle broadcasts the reciprocal row for normalization.
"""

import sys

sys.path.insert(0, "/opt/trn_rl_repo")

import numpy as np

import concourse.bass as bass
import concourse.mybir as mybir
import concourse.tile as tile
from concourse import bacc
from concourse.bass_utils import run_bass_kernel_spmd

B, S, QD = 2, 4096, 320
H, D = 8, 64
QDP = 384  # QD padded to 3 chunks of 128
F32 = mybir.dt.float32
F32R = mybir.dt.float32r


def build_nc(s=S, qb=512, trace_sim=False):
    """Build the per-core Bass program. s = sequence length (small for sim
    tests), qb = query block (columns of scores produced per PE/ACT round).
    """
    n_kt = s // 128  # key tiles
    n_qb = s // qb  # query blocks
    n_sb = max(s // 512, 1)  # 512-wide column blocks for the projections
    sbw = min(s, 512)
    n_qt = s // 128  # query tiles (output projection)
    assert qb % 512 == 0 or qb == s

    nc = bacc.Bacc("TRN2", target_bir_lowering=False, debug=False, num_devices=8)
    xT = nc.dram_tensor("xT", [QDP, s], F32R, kind="ExternalInput").ap()
    wq = nc.dram_tensor("wq", [QDP, 128], F32R, kind="ExternalInput").ap()
    wk = nc.dram_tensor("wk", [QDP, 128], F32R, kind="ExternalInput").ap()
    wv = nc.dram_tensor("wv", [QDP, 128], F32R, kind="ExternalInput").ap()
    wo = nc.dram_tensor("wo", [128, QD], F32R, kind="ExternalInput").ap()
    y = nc.dram_tensor("y", [s, QD], F32, kind="ExternalOutput").ap()

    with tile.TileContext(nc, trace_sim=trace_sim) as tc:
        with (
            tc.tile_pool(name="persist", bufs=1) as persist,
            tc.tile_pool(name="wpool", bufs=1) as wpool,
        ):
            # ---- warmup first ------------------------------------------------
            # bf16 warmup burst: dense PE work during the input DMAs so the
            # HAM clock-gate reaches K=8/8 (2.4 GHz) before the projections
            # start. fp32r keeps warmth but the cold-start is cheaper in bf16.
            with tc.tile_pool(name="warm_ps", bufs=1, space="PSUM") as warm_ps:
                wt = persist.tile([128, 512], mybir.dt.bfloat16, name="warm_t")
                nc.vector.memset(wt, 0.25)
                wp = warm_ps.tile([128, 512], F32)
                for i in range(24):
                    nc.tensor.matmul(
                        wp, wt[:, 0:128], wt, start=True, stop=True
                    )

            # ---- load inputs (weights first, x split per column block) ------
            w_sb = {}
            for name, w in (("wq", wq), ("wk", wk), ("wv", wv)):
                w_sb[name] = wpool.tile([128, 3, 128], F32R, name=f"{name}_sb")
                w_r = w.rearrange("(c p) m -> c p m", p=128)
                for c in range(3):
                    nc.sync.dma_start(out=w_sb[name][:, c, :], in_=w_r[c])
            wo_sb = wpool.tile([128, QD], F32R)
            nc.sync.dma_start(out=wo_sb, in_=wo)

            xT_sb = persist.tile([128, 3, s], F32R)
            xT_r = xT.rearrange("(c p) s -> c p s", p=128)
            for sb in range(n_sb):
                ssl = slice(sb * sbw, (sb + 1) * sbw)
                for c in range(3):
                    nc.sync.dma_start(out=xT_sb[:, c, ssl], in_=xT_r[c, :, ssl])

            qT_sb = persist.tile([128, s], F32R)  # rows 0:64 head A, 64:128 head B
            kT_sb = persist.tile([128, s], F32R)
            v_sb = persist.tile([128, 2, n_kt, 65], F32R)  # [k, head, ktile, d|1]
            outT_sb = persist.tile([128, s], F32R)  # normalized attn out, transposed

            for h in range(2):
                nc.vector.memset(v_sb[:, h, :, 64].bitcast(F32), 1.0)

            # ---- projections (per column block, dense PE pipeline) ----------
            with tc.tile_pool(name="proj_ps", bufs=3, space="PSUM") as proj_ps:
                for sb in range(n_sb):
                    ssl = slice(sb * sbw, (sb + 1) * sbw)
                    for name, dst in (("wk", kT_sb), ("wq", qT_sb)):
                        ps = proj_ps.tile([128, sbw], F32, name=f"ps_{name}", tag="qk")
                        for c in range(3):
                            nc.tensor.matmul(
                                ps,
                                w_sb[name][:, c, :],
                                xT_sb[:, c, ssl],
                                start=(c == 0),
                                stop=(c == 2),
                            )
                        nc.vector.tensor_copy(dst[:, ssl], ps)
                    for st in range(sb * (sbw // 128), (sb + 1) * (sbw // 128)):
                        vp = proj_ps.tile([128, 128], F32, tag="v")
                        for c in range(3):
                            nc.tensor.matmul(
                                vp,
                                xT_sb[:, c, st * 128 : (st + 1) * 128],
                                w_sb["wv"][:, c, :],
                                start=(c == 0),
                                stop=(c == 2),
                            )
                        for h in range(2):
                            nc.vector.tensor_copy(
                                v_sb[:, h, st, 0:64], vp[:, h * 64 : (h + 1) * 64]
                            )

            # ---- attention ---------------------------------------------------
            with (
                tc.tile_pool(name="sc_ps", bufs=2, space="PSUM") as sc_ps,
                tc.tile_pool(name="av_ps", bufs=2, space="PSUM") as av_ps,
                tc.tile_pool(name="exp_sb", bufs=3) as exp_pool,
                tc.tile_pool(name="norm_sb", bufs=2) as norm_pool,
            ):
                n_j = qb // 512 if qb >= 512 else 1
                jw = min(qb, 512)
                for q0 in range(n_qb):
                    qsl = slice(q0 * qb, (q0 + 1) * qb)
                    av = [
                        av_ps.tile([128, qb], F32, name=f"av{h}", tag=f"av{h}")
                        for h in range(2)
                    ]
                    for kt in range(n_kt):
                        sc = sc_ps.tile([128, 2 * qb], F32, tag="sc")
                        for h in range(2):
                            hp = slice(h * 64, (h + 1) * 64)
                            for j in range(n_j):
                                nc.tensor.matmul(
                                    sc[:, h * qb + j * jw : h * qb + (j + 1) * jw],
                                    kT_sb[hp, kt * 128 : (kt + 1) * 128],
                                    qT_sb[hp, q0 * qb + j * jw : q0 * qb + (j + 1) * jw],
                                    start=True,
                                    stop=True,
                                )
                        ex = exp_pool.tile([128, 2 * qb], F32R, tag="ex")
                        nc.scalar.activation(ex, sc, mybir.ActivationFunctionType.Exp)
                        for h in range(2):
                            for j in range(n_j):
                                nc.tensor.matmul(
                                    av[h][0:65, j * jw : (j + 1) * jw],
                                    v_sb[:, h, kt, :],
                                    ex[:, h * qb + j * jw : h * qb + (j + 1) * jw],
                                    start=(kt == 0),
                                    stop=(kt == n_kt - 1),
                                )
                    # normalize: out[d, q] = av[d, q] / av[64, q]
                    for h in range(2):
                        rrow = norm_pool.tile([1, qb], F32, tag="rrow")
                        nc.vector.reciprocal(rrow, av[h][64:65, :])
                        rc = norm_pool.tile([64, qb], F32, tag="rc")
                        nc.gpsimd.partition_broadcast(rc, rrow)
                        nc.vector.tensor_mul(
                            outT_sb[h * 64 : (h + 1) * 64, qsl], av[h][0:64, :], rc
                        )

            # ---- output projection ------------------------------------------
            with (
                tc.tile_pool(name="fin_ps", bufs=3, space="PSUM") as fin_ps,
                tc.tile_pool(name="y_sb", bufs=3) as y_pool,
            ):
                for qt in range(n_qt):
                    fp = fin_ps.tile([128, QD], F32, tag="fin")
                    qtsl = slice(qt * 128, (qt + 1) * 128)
                    # contract over all 128 partitions: rows 0:64 are head A's
                    # dims against Wo_A, 64:128 head B's against Wo_B -> the
                    # two heads' contributions sum in one matmul.
                    nc.tensor.matmul(
                        fp, outT_sb[:, qtsl], wo_sb, start=True, stop=True
                    )
                    yt = y_pool.tile([128, QD], F32, tag="y")
                    nc.vector.tensor_copy(yt, fp)
                    nc.sync.dma_start(out=y[qtsl, :], in_=yt)

    nc.compile()
    return nc


def host_inputs(x, Wq, Wk, Wv, Wo):
    """Build the 8 per-core input dicts from the full problem inputs."""
    scale = 1.0 / np.sqrt(np.float32(D))
    in_maps = []
    for core in range(8):
        b = core // 4
        h0 = 2 * (core % 4)
        cols = slice(h0 * D, (h0 + 2) * D)
        xT = np.zeros((QDP, S), np.float32)
        xT[:QD] = np.ascontiguousarray(x[b].T)
        wq_c = np.zeros((QDP, 128), np.float32)
        wq_c[:QD] = Wq[:, cols] * scale
        wk_c = np.zeros((QDP, 128), np.float32)
        wk_c[:QD] = Wk[:, cols]
        wv_c = np.zeros((QDP, 128), np.float32)
        wv_c[:QD] = Wv[:, cols]
        wo_c = np.ascontiguousarray(Wo[cols, :], dtype=np.float32)
        in_maps.append({"xT": xT, "wq": wq_c, "wk": wk_c, "wv": wv_c, "wo": wo_c})
    return in_maps


_NC_CACHE = {}


def kernel(x, Wq, Wk, Wv, Wo, bo, _trace=False, _trace_kwargs=None):
    x = np.asarray(x, np.float32)
    Wq = np.asarray(Wq, np.float32)
    Wk = np.asarray(Wk, np.float32)
    Wv = np.asarray(Wv, np.float32)
    Wo = np.asarray(Wo, np.float32)
    bo = np.asarray(bo, np.float32)

    if "nc" not in _NC_CACHE:
        _NC_CACHE["nc"] = build_nc()
    nc = _NC_CACHE["nc"]

    in_maps = host_inputs(x, Wq, Wk, Wv, Wo)
    kwargs = {}
    if _trace:
        kwargs = dict(trace=True, **(_trace_kwargs or {}))
    res = run_bass_kernel_spmd(nc, in_maps, core_ids=list(range(8)), **kwargs)

    y = np.zeros((B, S, QD), np.float32)
    for core in range(8):
        y[core // 4] += res.results[core]["y"]
    y += bo
    if _trace:
        return y, res
    return y
